# revision 1
# baseline (speedup 1.0000x reference)
"""ChebConv (K=3) kernel for Trainium2, data-parallel over batch across 8 NeuronCores.

Math (per batch b):
    d       = adj.sum(axis=1)                  (row sums)
    dinv    = (d + 1e-6) ** -0.5,  dsq = (d + 1e-6) ** 0.5
    M       = Dinv @ A @ Dinv      (so L = I - M)
    Tx0 = x, Tx1 = x - M x, Tx2 = 2(Tx1 - M Tx1) - x
    out     = relu(sum_k Txk @ W[k] + sum_k b[k])

Kernel-side reformulation (all scaled tensors, avoids materializing L):
    yk := Dinv @ Txk   (bf16 "weights" for the PE passes)
    y0 = Dinv x
    z1 = M x          (PE pass 1: lhsT = y0 tiles, rhs = scaled-transposed A)
    y1 = y0 - Dinv z1
    z2 = M Tx1        (PE pass 2: lhsT = y1 tiles)
    y2 = 2 y1 - y0 - 2 Dinv z2
    out = relu(Dsq @ (sum_k yk @ W[k]) + bsum)

A row-tiles stream in as fp32 over HWDGE; a single fused DVE/ACT op per tile
does the bf16 cast AND the row-sum reduction (accum_out). The transpose of A
that the PE needs is produced by per-tile matmuls against diag(dinv) as the
moving operand, which also applies the output-side Dinv scale:
ats2[j, i] = A[i, j] * dinv[i].  Pass 1 is emitted triangularly inside the
load loop so it overlaps the DMA stream.
"""

import numpy as np

B, N, F, K = 8, 2048, 128, 3
P = 128
NT = N // P  # 16
EPS = 1e-6
NCORES = 8

_cache = {}


def _build_nc():
    from contextlib import ExitStack

    import concourse.bacc as bacc
    import concourse.tile as tile
    from concourse import mybir

    f32 = mybir.dt.float32
    bf16 = mybir.dt.bfloat16
    AF = mybir.ActivationFunctionType
    OP = mybir.AluOpType

    nc = bacc.Bacc("TRN2", target_bir_lowering=False, debug=False, num_devices=NCORES)
    adj = nc.dram_tensor("adj", [N, N], f32, kind="ExternalInput").ap()
    x = nc.dram_tensor("x", [N, F], f32, kind="ExternalInput").ap()
    W = nc.dram_tensor("W", [K, F, F], f32, kind="ExternalInput").ap()
    bsum_d = nc.dram_tensor("bsum", [P, F], f32, kind="ExternalInput").ap()
    ident = nc.dram_tensor("ident", [P, P], f32, kind="ExternalInput").ap()
    out = nc.dram_tensor("out", [N, F], f32, kind="ExternalOutput").ap()
    out_t = out.rearrange("(t p) f -> p t f", p=P)

    with ExitStack() as ctx:
        tc = ctx.enter_context(tile.TileContext(nc))
        consts = ctx.enter_context(tc.tile_pool(name="consts", bufs=1))
        afp = ctx.enter_context(tc.tile_pool(name="afp", bufs=3))
        abp = ctx.enter_context(tc.tile_pool(name="abp", bufs=4))
        big = ctx.enter_context(tc.tile_pool(name="big", bufs=1))
        small = ctx.enter_context(tc.tile_pool(name="small", bufs=3))
        ps_acc = ctx.enter_context(tc.tile_pool(name="ps_acc", bufs=1, space="PSUM"))
        ps_t = ctx.enter_context(tc.tile_pool(name="ps_t", bufs=4, space="PSUM"))

        # ---- constants -------------------------------------------------
        ident_bf = consts.tile([P, P], bf16)
        nc.gpsimd.dma_start(out=ident_bf, in_=ident)
        w_bf = consts.tile([P, K, F], bf16)
        nc.gpsimd.dma_start(out=w_bf, in_=W.rearrange("k i o -> i k o"))
        bsum = consts.tile([P, F], f32)
        nc.sync.dma_start(out=bsum, in_=bsum_d)
        eps_sb = consts.tile([P, 1], f32)
        nc.vector.memset(eps_sb, EPS)

        # per-node scalars, [P, NT]: column r holds values for node tile r
        dsq = consts.tile([P, NT], f32)
        dinv = consts.tile([P, NT], f32)
        ndinv = consts.tile([P, NT], f32)
        n2dinv = consts.tile([P, NT], f32)

        y0 = big.tile([P, NT, F], bf16)
        y1 = big.tile([P, NT, F], bf16)
        y2 = big.tile([P, NT, F], bf16)
        tt = big.tile([P, NT, F], bf16)
        ats2 = big.tile([P, NT, N], bf16)  # [j_in_tile, c(j tile), i]: A[i,j]*dinv[i]
        yT = big.tile([P, K, N], bf16)     # transposed yk: [f, k, i]

        z1 = ps_acc.tile([P, N], f32, tag="acc")

        # ---- streaming phase: load A, fused cast+reduce, transpose,
        #      triangular pass-1 (overlaps the DMA stream) ----------------
        pt_y0 = None
        for r in range(NT):
            a_f = afp.tile([P, N], f32, tag="af")
            nc.sync.dma_start(out=a_f, in_=adj[r * P:(r + 1) * P, :])
            x_t = afp.tile([P, F], f32, tag="x")
            nc.sync.dma_start(out=x_t, in_=x[r * P:(r + 1) * P, :])

            # fused fp32->bf16 cast + row-sum (alternate DVE / ACT)
            a_t = abp.tile([P, N], bf16, tag="a")
            d_r = small.tile([P, 1], f32, tag="d")
            if r % 2 == 0:
                nc.vector.tensor_scalar(
                    out=a_t, in0=a_f, scalar1=1.0, scalar2=0.0, op0=OP.mult,
                    op1=OP.add, accum_out=d_r)
            else:
                nc.scalar.activation(out=a_t, in_=a_f, func=AF.Identity,
                                     accum_out=d_r)

            nc.scalar.activation(out=dsq[:, r:r + 1], in_=d_r, func=AF.Sqrt,
                                 bias=eps_sb)
            nc.vector.reciprocal(out=dinv[:, r:r + 1], in_=dsq[:, r:r + 1])
            nc.vector.tensor_scalar(out=ndinv[:, r:r + 1], in0=dinv[:, r:r + 1],
                                    scalar1=-1.0, scalar2=None, op0=OP.mult)
            nc.vector.tensor_scalar(out=n2dinv[:, r:r + 1], in0=dinv[:, r:r + 1],
                                    scalar1=-2.0, scalar2=None, op0=OP.mult)
            diag_r = small.tile([P, P], bf16, tag="diag")
            nc.vector.tensor_scalar(out=diag_r, in0=ident_bf,
                                    scalar1=dinv[:, r:r + 1], scalar2=None,
                                    op0=OP.mult)
            nc.vector.tensor_scalar(out=y0[:, r, :], in0=x_t,
                                    scalar1=dinv[:, r:r + 1], scalar2=None,
                                    op0=OP.mult)
            # transpose + dinv[i]-scale A tile row r: 16 (128x128) matmuls
            for g in range(4):
                pt = ps_t.tile([P, 4, P], f32, tag="t")
                for q in range(4):
                    c = 4 * g + q
                    nc.tensor.matmul(pt[:, q, :], lhsT=a_t[:, c * P:(c + 1) * P],
                                     rhs=diag_r, start=True, stop=True)
                if g % 2 == 0:
                    nc.vector.tensor_copy(
                        out=ats2[:, 4 * g:4 * g + 4, r * P:(r + 1) * P], in_=pt)
                else:
                    nc.scalar.copy(
                        out=ats2[:, 4 * g:4 * g + 4, r * P:(r + 1) * P], in_=pt)

            # transpose y0 tiles into yT[:, 0, :] once 4 are ready
            if r % 4 == 3:
                pt_y0 = ps_t.tile([P, 4, P], f32, tag="t")
                for q in range(4):
                    nc.tensor.matmul(pt_y0[:, q, :], lhsT=y0[:, r - 3 + q, :],
                                     rhs=ident_bf, start=True, stop=True)
                nc.scalar.copy(out=yT[:, 0, (r - 3) * P:(r + 1) * P], in_=pt_y0)

            # triangular pass-1 terms that became ready with tile r:
            # (a) strip r, weight blocks c <= r
            for c in range(r + 1):
                nc.tensor.matmul(z1[:, r * P:(r + 1) * P], lhsT=y0[:, c, :],
                                 rhs=ats2[:, c, r * P:(r + 1) * P],
                                 start=(r % 4 == 0 and c == 0),
                                 stop=(c == NT - 1), skip_group_check=True)
            # (b) older strips s < r with new weight block c = r (bank chunks)
            for sg in range((r + 3) // 4):
                lo = 4 * sg
                hi = min(lo + 4, r)  # strips [lo, hi)
                nc.tensor.matmul(z1[:, lo * P:hi * P], lhsT=y0[:, r, :],
                                 rhs=ats2[:, r, lo * P:hi * P],
                                 start=False, stop=(r == NT - 1),
                                 skip_group_check=True)

        # ---- recurrence + pass 2 --------------------------------------
        def z_to_nat_and_combine(zacc, zbf, scal, base, ydst):
            # zbf = bf16 cast of z (transposed layout); then per node tile:
            # ydst[r] = znat[r] * scal[r] + base[r]
            for nch in range(4):
                nc.any.tensor_copy(out=zbf[:, nch * 512:(nch + 1) * 512],
                                   in_=zacc[:, nch * 512:(nch + 1) * 512])
            for g in range(4):
                zn = ps_t.tile([P, 4, P], f32, tag="t")
                for q in range(4):
                    r = 4 * g + q
                    nc.tensor.matmul(zn[:, q, :], lhsT=zbf[:, r * P:(r + 1) * P],
                                     rhs=ident_bf, start=True, stop=True)
                for q in range(4):
                    r = 4 * g + q
                    nc.vector.scalar_tensor_tensor(
                        out=ydst[:, r, :], in0=zn[:, q, :], scalar=scal[:, r:r + 1],
                        in1=base[:, r, :], op0=OP.mult, op1=OP.add)

        def cheb_pass(weights, zacc):
            for c in range(NT):
                for nch in range(4):
                    nc.tensor.matmul(zacc[:, nch * 512:(nch + 1) * 512],
                                     lhsT=weights[:, c, :],
                                     rhs=ats2[:, c, nch * 512:(nch + 1) * 512],
                                     start=(c == 0), stop=(c == NT - 1))

        z1bf = big.tile([P, N], bf16, tag="zbf")
        z_to_nat_and_combine(z1, z1bf, ndinv, y0, y1)   # y1 = y0 - dinv*z1

        z2 = ps_acc.tile([P, N], f32, tag="acc")
        cheb_pass(y1, z2)

        # transpose y1 into yT[:, 1, :] and compute tt = 2*y1 - y0
        # (these only need y1; they overlap pass 2)
        for g in range(4):
            pt = ps_t.tile([P, 4, P], f32, tag="t")
            for q in range(4):
                r = 4 * g + q
                nc.tensor.matmul(pt[:, q, :], lhsT=y1[:, r, :], rhs=ident_bf,
                                 start=True, stop=True)
            nc.scalar.copy(out=yT[:, 1, g * 512:(g + 1) * 512], in_=pt)
        for r in range(NT):
            nc.vector.scalar_tensor_tensor(
                out=tt[:, r, :], in0=y1[:, r, :], scalar=2.0, in1=y0[:, r, :],
                op0=OP.mult, op1=OP.subtract)

        z2bf = big.tile([P, N], bf16, tag="zbf2")
        z_to_nat_and_combine(z2, z2bf, n2dinv, tt, y2)  # y2 = tt - 2*dinv*z2

        # ---- transpose y2, output layer -------------------------------
        for g in range(4):
            pt = ps_t.tile([P, 4, P], f32, tag="t")
            for q in range(4):
                r = 4 * g + q
                nc.tensor.matmul(pt[:, q, :], lhsT=y2[:, r, :], rhs=ident_bf,
                                 start=True, stop=True)
            nc.scalar.copy(out=yT[:, 2, g * 512:(g + 1) * 512], in_=pt)

        # out = relu(dsq * (sum_k yk @ Wk) + bsum)
        for g in range(4):
            og = small.tile([P, 4, F], f32, tag="og")
            for q in range(4):
                r = 4 * g + q
                oc = ps_t.tile([P, F], f32, tag="t")
                for k3 in range(K):
                    nc.tensor.matmul(oc, lhsT=yT[:, k3, r * P:(r + 1) * P],
                                     rhs=w_bf[:, k3, :],
                                     start=(k3 == 0), stop=(k3 == K - 1))
                tmp = small.tile([P, F], f32, tag="tmp")
                nc.vector.scalar_tensor_tensor(
                    out=tmp, in0=oc, scalar=dsq[:, r:r + 1], in1=bsum,
                    op0=OP.mult, op1=OP.add)
                nc.scalar.activation(out=og[:, q, :], in_=tmp, func=AF.Relu)
            nc.sync.dma_start(out=out_t[:, 4 * g:4 * g + 4, :], in_=og)

    nc.compile()
    return nc


def _get_nc():
    if "nc" not in _cache:
        _cache["nc"] = _build_nc()
    return _cache["nc"]


def make_in_maps(x, adj, W, b):
    ident = np.eye(P, dtype=np.float32)
    x = np.ascontiguousarray(np.asarray(x, dtype=np.float32))
    adj = np.ascontiguousarray(np.asarray(adj, dtype=np.float32))
    Wf = np.ascontiguousarray(np.asarray(W, dtype=np.float32))
    bf = np.asarray(b, dtype=np.float32)
    bsum = np.ascontiguousarray(
        np.broadcast_to(bf.sum(axis=0), (P, F)).astype(np.float32))
    return [
        {"adj": adj[c], "x": x[c], "W": Wf, "bsum": bsum, "ident": ident}
        for c in range(NCORES)
    ]


def run_raw(x, adj, W, b, **kwargs):
    from concourse import bass_utils

    nc = _get_nc()
    in_maps = make_in_maps(x, adj, W, b)
    res = bass_utils.run_bass_kernel_spmd(nc, in_maps,
                                          core_ids=list(range(NCORES)), **kwargs)
    out = np.stack([res.results[c]["out"] for c in range(NCORES)], axis=0)
    return out.astype(np.float32), res


def kernel(x, adj, W, b):
    out, _ = run_raw(x, adj, W, b)
    return out



# revision 4
# speedup vs baseline: 1.0604x; 1.0604x over previous
"""ChebConv (K=3) kernel for Trainium2, data-parallel over batch across 8 NeuronCores.

Math (per batch b):
    d    = adj.sum(axis=1)  (row sums), dinv = (d+eps)^-0.5, dsq = (d+eps)^0.5
    M    = Dinv A Dinv  (L = I - M)
    Tx0 = x, Tx1 = L x, Tx2 = 2 L Tx1 - Tx0
    out  = relu(sum_k Txk @ W[k] + sum_k b[k])

Kernel-side reformulation with y_k := Dinv Tx_k and ats3[j, i] = A[i, j]*dinv[i]^2
(the dinv^2 scale folds BOTH Dinv factors of every PE pass, so the Chebyshev
recurrence becomes pure elementwise subtracts in the transposed domain):
    y0  = Dinv x
    zt1 = pass(y0)  : zt1^T[f,i] = sum_j y0[j,f] ats3[j,i]  ( = (Dinv M x)^T )
    y1T = y0T - zt1^T                       (DVE subtract, no scaling)
    zt2 = pass(y1)                          ( = (Dinv M Tx1)^T )
    y2T = 2 y1T - y0T - 2 zt2^T             (DVE)
    out = relu(Dsq @ (sum_k y_k @ W[k]) + bsum)   with y_k @ W via lhsT = yT_k

A row-strips stream as fp32 in two half-strip DMAs; DVE casts+row-sums the low
half while ACT does the high half (fused accum_out). The scaled transpose of A
is produced per-tile by PE matmuls against diag(dinv^2); pass 1 is emitted
triangularly inside the load loop so it overlaps the DMA stream. The output
layer reads the transposed yT_k directly (lhsT = yT_k block, rhs = W[k]).
"""

import numpy as np

B, N, F, K = 8, 2048, 128, 3
P = 128
NT = N // P  # 16
H = N // 2   # 1024
EPS = 1e-6
NCORES = 8

_cache = {}


def _build_nc():
    from contextlib import ExitStack

    import concourse.bacc as bacc
    import concourse.tile as tile
    from concourse import mybir

    f32 = mybir.dt.float32
    bf16 = mybir.dt.bfloat16
    AF = mybir.ActivationFunctionType
    OP = mybir.AluOpType

    nc = bacc.Bacc("TRN2", target_bir_lowering=False, debug=False, num_devices=NCORES)
    adj = nc.dram_tensor("adj", [N, N], f32, kind="ExternalInput").ap()
    x = nc.dram_tensor("x", [N, F], f32, kind="ExternalInput").ap()
    W = nc.dram_tensor("W", [K, F, F], f32, kind="ExternalInput").ap()
    bsum_d = nc.dram_tensor("bsum", [P, F], f32, kind="ExternalInput").ap()
    ident = nc.dram_tensor("ident", [P, P], f32, kind="ExternalInput").ap()
    out = nc.dram_tensor("out", [N, F], f32, kind="ExternalOutput").ap()
    out_t = out.rearrange("(t p) f -> p t f", p=P)

    with ExitStack() as ctx:
        tc = ctx.enter_context(tile.TileContext(nc))
        consts = ctx.enter_context(tc.tile_pool(name="consts", bufs=1))
        afp = ctx.enter_context(tc.tile_pool(name="afp", bufs=4))
        abp = ctx.enter_context(tc.tile_pool(name="abp", bufs=3))
        dgp = ctx.enter_context(tc.tile_pool(name="dgp", bufs=2))
        big = ctx.enter_context(tc.tile_pool(name="big", bufs=1))
        small = ctx.enter_context(tc.tile_pool(name="small", bufs=3))
        ps_acc = ctx.enter_context(tc.tile_pool(name="ps_acc", bufs=1, space="PSUM"))
        ps_t = ctx.enter_context(tc.tile_pool(name="ps_t", bufs=4, space="PSUM"))

        # ---- constants -------------------------------------------------
        ident_bf = consts.tile([P, P], bf16)
        nc.gpsimd.dma_start(out=ident_bf, in_=ident)
        w_bf = consts.tile([P, K, F], bf16)
        nc.gpsimd.dma_start(out=w_bf, in_=W.rearrange("k i o -> i k o"))
        bsum = consts.tile([P, F], f32)
        nc.scalar.dma_start(out=bsum, in_=bsum_d)
        x_sb = consts.tile([P, NT, F], f32)
        nc.scalar.dma_start(out=x_sb, in_=x.rearrange("(t p) f -> p t f", p=P))
        eps_sb = consts.tile([P, 1], f32)
        nc.vector.memset(eps_sb, EPS)

        # per-node scalars, [P, NT]: column r holds values for node tile r
        dsq = consts.tile([P, NT], f32)
        dinv = consts.tile([P, NT], f32)
        dinv2 = consts.tile([P, NT], f32)

        y0 = big.tile([P, NT, F], bf16)    # Dinv x, natural (pass-1 lhsT)
        y1n = big.tile([P, NT, F], bf16)   # y1 natural (pass-2 lhsT)
        ats3 = big.tile([P, NT, N], bf16)  # [j_in_tile, c(j tile), i]: A[i,j]*dinv2[i]
        yT0 = big.tile([P, N], bf16)       # transposed y0: [f, i]
        yT1 = big.tile([P, N], bf16)
        yT2 = big.tile([P, N], bf16)
        ttT = big.tile([P, N], bf16)       # 2*y1T - y0T

        z1 = ps_acc.tile([P, N], f32, tag="acc")

        # ---- streaming phase: load A (two half-strips), fused cast+reduce
        #      on DVE+ACT in parallel, transpose w/ dinv^2 scale,
        #      triangular pass-1 (overlaps the DMA stream) ----------------
        for r in range(NT):
            a_f = afp.tile([P, N], f32, tag="af")
            nc.sync.dma_start(out=a_f[:, :H], in_=adj[r * P:(r + 1) * P, :H])
            nc.sync.dma_start(out=a_f[:, H:], in_=adj[r * P:(r + 1) * P, H:])

            # fused fp32->bf16 cast + row-sum halves (DVE low, ACT high)
            a_t = abp.tile([P, N], bf16, tag="a")
            d_lo = small.tile([P, 1], f32, tag="dlo")
            d_hi = small.tile([P, 1], f32, tag="dhi")
            nc.vector.tensor_scalar(
                out=a_t[:, :H], in0=a_f[:, :H], scalar1=1.0, scalar2=0.0,
                op0=OP.mult, op1=OP.add, accum_out=d_lo)
            nc.scalar.activation(out=a_t[:, H:], in_=a_f[:, H:],
                                 func=AF.Identity, accum_out=d_hi)

            d_r = small.tile([P, 1], f32, tag="d")
            nc.vector.tensor_scalar(out=d_r, in0=d_lo, scalar1=d_hi,
                                    scalar2=None, op0=OP.add)
            nc.scalar.activation(out=dsq[:, r:r + 1], in_=d_r, func=AF.Sqrt,
                                 bias=eps_sb)
            nc.vector.reciprocal(out=dinv[:, r:r + 1], in_=dsq[:, r:r + 1])
            nc.vector.tensor_scalar(out=dinv2[:, r:r + 1], in0=dinv[:, r:r + 1],
                                    scalar1=dinv[:, r:r + 1], scalar2=None,
                                    op0=OP.mult)
            diag2 = dgp.tile([P, P], bf16, tag="diag")
            nc.vector.tensor_scalar(out=diag2, in0=ident_bf,
                                    scalar1=dinv2[:, r:r + 1], scalar2=None,
                                    op0=OP.mult)
            nc.vector.tensor_scalar(out=y0[:, r, :], in0=x_sb[:, r, :],
                                    scalar1=dinv[:, r:r + 1], scalar2=None,
                                    op0=OP.mult)
            # y0T strip r (transpose y0 via PE)
            pt0 = ps_t.tile([P, 4, P], f32, tag="t")
            nc.tensor.matmul(pt0[:, 0, :], lhsT=y0[:, r, :], rhs=ident_bf,
                             start=True, stop=True)
            nc.scalar.copy(out=yT0[:, r * P:(r + 1) * P], in_=pt0[:, 0, :])

            # transpose + dinv2[i]-scale A strip r: 16 (128x128) matmuls
            for g in range(4):
                pt = ps_t.tile([P, 4, P], f32, tag="t")
                for q in range(4):
                    c = 4 * g + q
                    nc.tensor.matmul(pt[:, q, :], lhsT=a_t[:, c * P:(c + 1) * P],
                                     rhs=diag2, start=True, stop=True)
                if g % 2 == 0:
                    nc.vector.tensor_copy(
                        out=ats3[:, 4 * g:4 * g + 4, r * P:(r + 1) * P], in_=pt)
                else:
                    nc.scalar.copy(
                        out=ats3[:, 4 * g:4 * g + 4, r * P:(r + 1) * P], in_=pt)

            # triangular pass-1 terms that became ready with strip r:
            # (a) strip r, weight blocks c <= r
            for c in range(r + 1):
                nc.tensor.matmul(z1[:, r * P:(r + 1) * P], lhsT=y0[:, c, :],
                                 rhs=ats3[:, c, r * P:(r + 1) * P],
                                 start=(r % 4 == 0 and c == 0),
                                 stop=(c == NT - 1), skip_group_check=True)
            # (b) older strips s < r with new weight block c = r (bank chunks)
            for sg in range((r + 3) // 4):
                lo = 4 * sg
                hi = min(lo + 4, r)  # strips [lo, hi)
                nc.tensor.matmul(z1[:, lo * P:hi * P], lhsT=y0[:, r, :],
                                 rhs=ats3[:, r, lo * P:hi * P],
                                 start=False, stop=(r == NT - 1),
                                 skip_group_check=True)

        # ---- recurrence: y1T = y0T - z1T (pure DVE, no scaling) --------
        for ch in range(4):
            s = slice(ch * 512, (ch + 1) * 512)
            nc.vector.scalar_tensor_tensor(
                out=yT1[:, s], in0=z1[:, s], scalar=-1.0, in1=yT0[:, s],
                op0=OP.mult, op1=OP.add)

        # ttT = 2*y1T - y0T (overlaps pass 2)
        for ch in range(4):
            s = slice(ch * 512, (ch + 1) * 512)
            nc.vector.scalar_tensor_tensor(
                out=ttT[:, s], in0=yT1[:, s], scalar=2.0, in1=yT0[:, s],
                op0=OP.mult, op1=OP.subtract)

        # ---- pass 2: y1 natural per block (transpose of y1T), wide MMs -
        z2 = ps_acc.tile([P, N], f32, tag="acc")
        for c in range(NT):
            ptc = ps_t.tile([P, 4, P], f32, tag="t")
            nc.tensor.matmul(ptc[:, 0, :], lhsT=yT1[:, c * P:(c + 1) * P],
                             rhs=ident_bf, start=True, stop=True)
            nc.scalar.copy(out=y1n[:, c, :], in_=ptc[:, 0, :])
            for nch in range(4):
                nc.tensor.matmul(z2[:, nch * 512:(nch + 1) * 512],
                                 lhsT=y1n[:, c, :],
                                 rhs=ats3[:, c, nch * 512:(nch + 1) * 512],
                                 start=(c == 0), stop=(c == NT - 1))

        # y2T = ttT - 2*z2T (DVE)
        for ch in range(4):
            s = slice(ch * 512, (ch + 1) * 512)
            nc.vector.scalar_tensor_tensor(
                out=yT2[:, s], in0=z2[:, s], scalar=-2.0, in1=ttT[:, s],
                op0=OP.mult, op1=OP.add)

        # ---- output layer: out[i,:] = relu(dsq[i]*(sum_k yTk[:,i]^T Wk)+bsum)
        yTs = (yT0, yT1, yT2)
        for g in range(4):
            og = small.tile([P, 4, F], f32, tag="og")
            for q in range(4):
                rr = 4 * g + q
                oc = ps_t.tile([P, 4, P], f32, tag="t")
                for k3 in range(K):
                    nc.tensor.matmul(oc[:, 0, :],
                                     lhsT=yTs[k3][:, rr * P:(rr + 1) * P],
                                     rhs=w_bf[:, k3, :],
                                     start=(k3 == 0), stop=(k3 == K - 1))
                tmp = small.tile([P, F], f32, tag="tmp")
                nc.vector.scalar_tensor_tensor(
                    out=tmp, in0=oc[:, 0, :], scalar=dsq[:, rr:rr + 1], in1=bsum,
                    op0=OP.mult, op1=OP.add)
                nc.scalar.activation(out=og[:, q, :], in_=tmp, func=AF.Relu)
            nc.sync.dma_start(out=out_t[:, 4 * g:4 * g + 4, :], in_=og)

    nc.compile()
    return nc


def _get_nc():
    if "nc" not in _cache:
        _cache["nc"] = _build_nc()
    return _cache["nc"]


def make_in_maps(x, adj, W, b):
    ident = np.eye(P, dtype=np.float32)
    x = np.ascontiguousarray(np.asarray(x, dtype=np.float32))
    adj = np.ascontiguousarray(np.asarray(adj, dtype=np.float32))
    Wf = np.ascontiguousarray(np.asarray(W, dtype=np.float32))
    bf = np.asarray(b, dtype=np.float32)
    bsum = np.ascontiguousarray(
        np.broadcast_to(bf.sum(axis=0), (P, F)).astype(np.float32))
    return [
        {"adj": adj[c], "x": x[c], "W": Wf, "bsum": bsum, "ident": ident}
        for c in range(NCORES)
    ]


def run_raw(x, adj, W, b, **kwargs):
    from concourse import bass_utils

    nc = _get_nc()
    in_maps = make_in_maps(x, adj, W, b)
    res = bass_utils.run_bass_kernel_spmd(nc, in_maps,
                                          core_ids=list(range(NCORES)), **kwargs)
    out = np.stack([res.results[c]["out"] for c in range(NCORES)], axis=0)
    return out.astype(np.float32), res


def kernel(x, adj, W, b):
    out, _ = run_raw(x, adj, W, b)
    return out


# revision 5
# speedup vs baseline: 1.0646x; 1.0040x over previous
"""ChebConv (K=3) kernel for Trainium2, data-parallel over batch across 8 NeuronCores.

Math (per batch b):
    d    = adj.sum(axis=1)  (row sums), dinv = (d+eps)^-0.5, dsq = (d+eps)^0.5
    M    = Dinv A Dinv  (L = I - M)
    Tx0 = x, Tx1 = L x, Tx2 = 2 L Tx1 - Tx0
    out  = relu(sum_k Txk @ W[k] + sum_k b[k])

Kernel-side reformulation with u_k := Dsq Tx_k and UNSCALED at2[j, i] = A[i, j]:
every PE pass contracts a Dinv^2-scaled natural operand against plain A^T, so
the transpose of A needs no scaling (it only depends on the bf16 cast, not the
row-sum chain) and the Chebyshev recurrence is elementwise in the transposed
domain:
    u0  = Dsq x,  yp0 = Dinv x  (= Dinv^2 u0)
    z1T = pass(yp0) : z1T[f,i] = sum_j yp0[j,f] at2[j,i]   ( = (Dsq M x)^T )
    u1T = u0T - z1T                                  (DVE subtract)
    y1s[c] = dinv2[c] * u1n[c]   (scale fused into the ACT PSUM->SBUF copy)
    z2T = pass(y1s)                                  ( = (Dsq M Tx1)^T )
    u2T = 2 u1T - u0T - 2 z2T                        (DVE)
    out = relu(Dinv @ (sum_k u_k @ W[k]) + bsum)     (lhsT = u_kT blocks)

A row-strips stream as fp32 in two half-strip DMAs; DVE casts+row-sums the low
half while ACT does the high half (fused accum_out). ident/W are preloaded as
bf16 from the host over HWDGE (no slow gpsimd cast-DMA). Pass 1 is emitted
triangularly inside the load loop so it overlaps the DMA stream.
"""

import numpy as np

B, N, F, K = 8, 2048, 128, 3
P = 128
NT = N // P  # 16
H = N // 2   # 1024
EPS = 1e-6
NCORES = 8

_cache = {}


def _build_nc():
    from contextlib import ExitStack

    import concourse.bacc as bacc
    import concourse.tile as tile
    from concourse import mybir

    f32 = mybir.dt.float32
    bf16 = mybir.dt.bfloat16
    AF = mybir.ActivationFunctionType
    OP = mybir.AluOpType

    nc = bacc.Bacc("TRN2", target_bir_lowering=False, debug=False, num_devices=NCORES)
    adj = nc.dram_tensor("adj", [N, N], f32, kind="ExternalInput").ap()
    x = nc.dram_tensor("x", [N, F], f32, kind="ExternalInput").ap()
    wb_d = nc.dram_tensor("wb", [P, K, F], bf16, kind="ExternalInput").ap()
    bsum_d = nc.dram_tensor("bsum", [P, F], f32, kind="ExternalInput").ap()
    identb_d = nc.dram_tensor("identb", [P, P], bf16, kind="ExternalInput").ap()
    out = nc.dram_tensor("out", [N, F], f32, kind="ExternalOutput").ap()
    out_t = out.rearrange("(t p) f -> p t f", p=P)

    with ExitStack() as ctx:
        tc = ctx.enter_context(tile.TileContext(nc))
        consts = ctx.enter_context(tc.tile_pool(name="consts", bufs=1))
        afp = ctx.enter_context(tc.tile_pool(name="afp", bufs=4))
        abp = ctx.enter_context(tc.tile_pool(name="abp", bufs=3))
        big = ctx.enter_context(tc.tile_pool(name="big", bufs=1))
        small = ctx.enter_context(tc.tile_pool(name="small", bufs=3))
        ps_acc = ctx.enter_context(tc.tile_pool(name="ps_acc", bufs=1, space="PSUM"))
        ps_t = ctx.enter_context(tc.tile_pool(name="ps_t", bufs=4, space="PSUM"))

        # ---- constants (all HWDGE, no casting DMA) ---------------------
        ident_bf = consts.tile([P, P], bf16)
        nc.sync.dma_start(out=ident_bf, in_=identb_d)
        w_bf = consts.tile([P, K, F], bf16)
        nc.sync.dma_start(out=w_bf, in_=wb_d)
        bsum = consts.tile([P, F], f32)
        nc.scalar.dma_start(out=bsum, in_=bsum_d)
        x_sb = consts.tile([P, NT, F], f32)
        nc.scalar.dma_start(out=x_sb, in_=x.rearrange("(t p) f -> p t f", p=P))
        eps_sb = consts.tile([P, 1], f32)
        nc.vector.memset(eps_sb, EPS)

        # per-node scalars, [P, NT]: column r holds values for node tile r
        dinv = consts.tile([P, NT], f32)
        dinv2 = consts.tile([P, NT], f32)

        u0 = big.tile([P, NT, F], bf16)    # Dsq x, natural (u0T source)
        yp0 = big.tile([P, NT, F], bf16)   # Dinv x, natural (pass-1 lhsT)
        y1s = big.tile([P, NT, F], bf16)   # dinv2 * u1, natural (pass-2 lhsT)
        at2 = big.tile([P, NT, N], bf16)   # [j_in_tile, c(j tile), i]: A[i,j]
        uT0 = big.tile([P, N], bf16)       # transposed u0: [f, i]
        uT1 = big.tile([P, N], bf16)
        uT2 = big.tile([P, N], bf16)
        ttT = big.tile([P, N], bf16)       # 2*u1T - u0T

        z1 = ps_acc.tile([P, N], f32, tag="acc")

        # ---- streaming phase -------------------------------------------
        for r in range(NT):
            a_f = afp.tile([P, N], f32, tag="af")
            nc.sync.dma_start(out=a_f[:, :H], in_=adj[r * P:(r + 1) * P, :H])
            nc.sync.dma_start(out=a_f[:, H:], in_=adj[r * P:(r + 1) * P, H:])

            # fused fp32->bf16 cast + row-sum halves (DVE low, ACT high)
            a_t = abp.tile([P, N], bf16, tag="a")
            d_lo = small.tile([P, 1], f32, tag="dlo")
            d_hi = small.tile([P, 1], f32, tag="dhi")
            nc.vector.tensor_scalar(
                out=a_t[:, :H], in0=a_f[:, :H], scalar1=1.0, scalar2=0.0,
                op0=OP.mult, op1=OP.add, accum_out=d_lo)
            nc.scalar.activation(out=a_t[:, H:], in_=a_f[:, H:],
                                 func=AF.Identity, accum_out=d_hi)

            # transpose A strip r (no scaling -> only gated on the cast)
            for g in range(4):
                pt = ps_t.tile([P, 4, P], f32, tag="t")
                for q in range(4):
                    c = 4 * g + q
                    nc.tensor.matmul(pt[:, q, :], lhsT=a_t[:, c * P:(c + 1) * P],
                                     rhs=ident_bf, start=True, stop=True)
                if g % 2 == 0:
                    nc.vector.tensor_copy(
                        out=at2[:, 4 * g:4 * g + 4, r * P:(r + 1) * P], in_=pt)
                else:
                    nc.scalar.copy(
                        out=at2[:, 4 * g:4 * g + 4, r * P:(r + 1) * P], in_=pt)

            # scalar chain for strip r
            d_r = small.tile([P, 1], f32, tag="d")
            nc.vector.tensor_scalar(out=d_r, in0=d_lo, scalar1=d_hi,
                                    scalar2=None, op0=OP.add)
            dsq_r = small.tile([P, 1], f32, tag="dsq")
            nc.scalar.activation(out=dsq_r, in_=d_r, func=AF.Sqrt, bias=eps_sb)
            nc.vector.reciprocal(out=dinv[:, r:r + 1], in_=dsq_r)
            nc.vector.tensor_scalar(out=dinv2[:, r:r + 1], in0=dinv[:, r:r + 1],
                                    scalar1=dinv[:, r:r + 1], scalar2=None,
                                    op0=OP.mult)
            nc.vector.tensor_scalar(out=u0[:, r, :], in0=x_sb[:, r, :],
                                    scalar1=dsq_r, scalar2=None, op0=OP.mult)
            nc.vector.tensor_scalar(out=yp0[:, r, :], in0=x_sb[:, r, :],
                                    scalar1=dinv[:, r:r + 1], scalar2=None,
                                    op0=OP.mult)
            # u0T strip r (transpose u0 via PE)
            pt0 = ps_t.tile([P, 4, P], f32, tag="t")
            nc.tensor.matmul(pt0[:, 0, :], lhsT=u0[:, r, :], rhs=ident_bf,
                             start=True, stop=True)
            nc.scalar.copy(out=uT0[:, r * P:(r + 1) * P], in_=pt0[:, 0, :])

            # triangular pass-1 terms that became ready with strip r:
            # (a) older strips s < r with new weight block c = r (bank chunks)
            #     -- only needs yp0[r] + old transposes, so emit it first
            for sg in range((r + 3) // 4):
                lo = 4 * sg
                hi = min(lo + 4, r)  # strips [lo, hi)
                nc.tensor.matmul(z1[:, lo * P:hi * P], lhsT=yp0[:, r, :],
                                 rhs=at2[:, r, lo * P:hi * P],
                                 start=False, stop=(r == NT - 1),
                                 skip_group_check=True)
            # (b) strip r, weight blocks c <= r
            for c in range(r + 1):
                nc.tensor.matmul(z1[:, r * P:(r + 1) * P], lhsT=yp0[:, c, :],
                                 rhs=at2[:, c, r * P:(r + 1) * P],
                                 start=(r % 4 == 0 and c == 0),
                                 stop=(c == NT - 1), skip_group_check=True)

        # ---- recurrence: u1T = u0T - z1T (pure DVE, no scaling) --------
        for ch in range(4):
            s = slice(ch * 512, (ch + 1) * 512)
            nc.vector.scalar_tensor_tensor(
                out=uT1[:, s], in0=z1[:, s], scalar=-1.0, in1=uT0[:, s],
                op0=OP.mult, op1=OP.add)

        # ttT = 2*u1T - u0T (overlaps pass 2)
        for ch in range(4):
            s = slice(ch * 512, (ch + 1) * 512)
            nc.vector.scalar_tensor_tensor(
                out=ttT[:, s], in0=uT1[:, s], scalar=2.0, in1=uT0[:, s],
                op0=OP.mult, op1=OP.subtract)

        # ---- pass 2: y1s[c] = dinv2[c]*u1n[c] (scale fused in ACT copy),
        #      then wide MMs against at2 ---------------------------------
        z2 = ps_acc.tile([P, N], f32, tag="acc")
        for c in range(NT):
            ptc = ps_t.tile([P, 4, P], f32, tag="t")
            nc.tensor.matmul(ptc[:, 0, :], lhsT=uT1[:, c * P:(c + 1) * P],
                             rhs=ident_bf, start=True, stop=True)
            nc.scalar.mul(out=y1s[:, c, :], in_=ptc[:, 0, :],
                          mul=dinv2[:, c:c + 1])
            for nch in range(4):
                nc.tensor.matmul(z2[:, nch * 512:(nch + 1) * 512],
                                 lhsT=y1s[:, c, :],
                                 rhs=at2[:, c, nch * 512:(nch + 1) * 512],
                                 start=(c == 0), stop=(c == NT - 1))

        # u2T = ttT - 2*z2T (DVE)
        for ch in range(4):
            s = slice(ch * 512, (ch + 1) * 512)
            nc.vector.scalar_tensor_tensor(
                out=uT2[:, s], in0=z2[:, s], scalar=-2.0, in1=ttT[:, s],
                op0=OP.mult, op1=OP.add)

        # ---- output layer: out[i,:] = relu(dinv[i]*(sum_k uTk[:,i]^T Wk)+bsum)
        uTs = (uT0, uT1, uT2)
        for g in range(4):
            og = small.tile([P, 4, F], f32, tag="og")
            for q in range(4):
                rr = 4 * g + q
                oc = ps_t.tile([P, 4, P], f32, tag="t")
                for k3 in range(K):
                    nc.tensor.matmul(oc[:, 0, :],
                                     lhsT=uTs[k3][:, rr * P:(rr + 1) * P],
                                     rhs=w_bf[:, k3, :],
                                     start=(k3 == 0), stop=(k3 == K - 1))
                tmp = small.tile([P, F], f32, tag="tmp")
                nc.vector.scalar_tensor_tensor(
                    out=tmp, in0=oc[:, 0, :], scalar=dinv[:, rr:rr + 1],
                    in1=bsum, op0=OP.mult, op1=OP.add)
                nc.scalar.activation(out=og[:, q, :], in_=tmp, func=AF.Relu)
            nc.sync.dma_start(out=out_t[:, 4 * g:4 * g + 4, :], in_=og)

    nc.compile()
    return nc


def _get_nc():
    if "nc" not in _cache:
        _cache["nc"] = _build_nc()
    return _cache["nc"]


def make_in_maps(x, adj, W, b):
    import ml_dtypes

    bf16 = ml_dtypes.bfloat16
    identb = np.ascontiguousarray(np.eye(P, dtype=np.float32).astype(bf16))
    x = np.ascontiguousarray(np.asarray(x, dtype=np.float32))
    adj = np.ascontiguousarray(np.asarray(adj, dtype=np.float32))
    # W [K, in, out] -> [in, K, out] bf16 (host-side rearrange + cast)
    wb = np.ascontiguousarray(
        np.asarray(W, dtype=np.float32).transpose(1, 0, 2).astype(bf16))
    bf = np.asarray(b, dtype=np.float32)
    bsum = np.ascontiguousarray(
        np.broadcast_to(bf.sum(axis=0), (P, F)).astype(np.float32))
    return [
        {"adj": adj[c], "x": x[c], "wb": wb, "bsum": bsum, "identb": identb}
        for c in range(NCORES)
    ]


def run_raw(x, adj, W, b, **kwargs):
    from concourse import bass_utils

    nc = _get_nc()
    in_maps = make_in_maps(x, adj, W, b)
    res = bass_utils.run_bass_kernel_spmd(nc, in_maps,
                                          core_ids=list(range(NCORES)), **kwargs)
    out = np.stack([res.results[c]["out"] for c in range(NCORES)], axis=0)
    return out.astype(np.float32), res


def kernel(x, adj, W, b):
    out, _ = run_raw(x, adj, W, b)
    return out


# revision 8
# speedup vs baseline: 1.1124x; 1.0449x over previous
"""ChebConv (K=3) kernel for Trainium2, data-parallel over batch across 8 NeuronCores.

Math (per batch b):
    d    = adj.sum(axis=1)  (row sums), dinv = (d+eps)^-0.5, dsq = (d+eps)^0.5
    M    = Dinv A Dinv  (L = I - M)
    Tx0 = x, Tx1 = L x, Tx2 = 2 L Tx1 - Tx0
    out  = relu(sum_k Txk @ W[k] + sum_k b[k])

Kernel-side reformulation with u_k := Dsq Tx_k and UNSCALED at2[j, i] = A[i, j]:
every PE pass contracts a Dinv^2-scaled natural operand against plain A^T, so
the transpose of A needs no scaling (it only depends on the bf16 cast, not the
row-sum chain) and the Chebyshev recurrence is elementwise in the transposed
domain:
    u0  = Dsq x,  yp0 = Dinv x  (= Dinv^2 u0)
    z1T = pass(yp0) : z1T[f,i] = sum_j yp0[j,f] at2[j,i]   ( = (Dsq M x)^T )
    u1T = u0T - z1T                                  (DVE subtract)
    y1s[c] = dinv2[c] * u1n[c]   (scale fused into the ACT PSUM->SBUF copy)
    z2T = pass(y1s)                                  ( = (Dsq M Tx1)^T )
    u2T = 2 u1T - u0T - 2 z2T                        (DVE)
    out = relu(Dinv @ (sum_k u_k @ W[k]) + bsum)     (lhsT = u_kT blocks)

A row-strips stream as fp32 in two half-strip DMAs; DVE casts+row-sums the low
half while ACT does the high half (fused accum_out). ident/W are preloaded as
bf16 from the host over HWDGE (no slow gpsimd cast-DMA). Pass 1 is emitted
triangularly inside the load loop so it overlaps the DMA stream.
"""

import numpy as np

B, N, F, K = 8, 2048, 128, 3
P = 128
NT = N // P  # 16
H = N // 2   # 1024
EPS = 1e-6
NCORES = 8

_cache = {}


def _build_nc():
    from contextlib import ExitStack

    import concourse.bacc as bacc
    import concourse.tile as tile
    from concourse import mybir

    f32 = mybir.dt.float32
    bf16 = mybir.dt.bfloat16
    AF = mybir.ActivationFunctionType
    OP = mybir.AluOpType

    nc = bacc.Bacc("TRN2", target_bir_lowering=False, debug=False, num_devices=NCORES)
    adj = nc.dram_tensor("adj", [N, N], f32, kind="ExternalInput").ap()
    x = nc.dram_tensor("x", [N, F], f32, kind="ExternalInput").ap()
    wb_d = nc.dram_tensor("wb", [P, K, F], bf16, kind="ExternalInput").ap()
    bsum_d = nc.dram_tensor("bsum", [P, F], f32, kind="ExternalInput").ap()
    identb_d = nc.dram_tensor("identb", [P, P], bf16, kind="ExternalInput").ap()
    out = nc.dram_tensor("out", [N, F], f32, kind="ExternalOutput").ap()
    out_t = out.rearrange("(t p) f -> p t f", p=P)

    with ExitStack() as ctx:
        tc = ctx.enter_context(tile.TileContext(nc))
        consts = ctx.enter_context(tc.tile_pool(name="consts", bufs=1))
        afp = ctx.enter_context(tc.tile_pool(name="afp", bufs=5))
        abp = ctx.enter_context(tc.tile_pool(name="abp", bufs=4))
        big = ctx.enter_context(tc.tile_pool(name="big", bufs=1))
        small = ctx.enter_context(tc.tile_pool(name="small", bufs=3))
        ps_acc = ctx.enter_context(tc.tile_pool(name="ps_acc", bufs=1, space="PSUM"))
        ps_t = ctx.enter_context(tc.tile_pool(name="ps_t", bufs=4, space="PSUM"))

        # ---- constants (all HWDGE, no casting DMA) ---------------------
        ident_bf = consts.tile([P, P], bf16)
        nc.sync.dma_start(out=ident_bf, in_=identb_d)
        w_bf = consts.tile([P, K, F], bf16)
        nc.sync.dma_start(out=w_bf, in_=wb_d)
        bsum = consts.tile([P, F], f32)
        nc.scalar.dma_start(out=bsum, in_=bsum_d)
        x_sb = consts.tile([P, NT, F], f32)
        nc.scalar.dma_start(out=x_sb, in_=x.rearrange("(t p) f -> p t f", p=P))
        eps_sb = consts.tile([P, 1], f32)
        nc.vector.memset(eps_sb, EPS)

        # per-node scalars, [P, NT]: column r holds values for node tile r
        dinv = consts.tile([P, NT], f32)
        dinv2 = consts.tile([P, NT], f32)

        u0 = big.tile([P, NT, F], bf16)    # Dsq x, natural (u0T source)
        yp0 = big.tile([P, NT, F], bf16)   # Dinv x, natural (pass-1 lhsT)
        y1s = big.tile([P, NT, F], bf16)   # dinv2 * u1, natural (pass-2 lhsT)
        at2 = big.tile([P, NT, N], bf16)   # [j_in_tile, c(j tile), i]: A[i,j]
        uT0 = big.tile([P, N], bf16)       # transposed u0: [f, i]
        uT1 = big.tile([P, N], bf16)
        uT2 = big.tile([P, N], bf16)
        ttT = big.tile([P, N], bf16)       # 2*u1T - u0T

        z1 = ps_acc.tile([P, N], f32, tag="acc")

        def emit_u0T_and_pass1(r):
            # u0T strip r (transpose u0 via PE)
            pt0 = ps_t.tile([P, 4, P], f32, tag="t")
            nc.tensor.matmul(pt0[:, 0, :], lhsT=u0[:, r, :], rhs=ident_bf,
                             start=True, stop=True)
            nc.scalar.copy(out=uT0[:, r * P:(r + 1) * P], in_=pt0[:, 0, :])
            # triangular pass-1 terms that became ready with strip r:
            # (a) older strips s < r with new weight block c = r (bank chunks)
            for sg in range((r + 3) // 4):
                lo = 4 * sg
                hi = min(lo + 4, r)  # strips [lo, hi)
                nc.tensor.matmul(z1[:, lo * P:hi * P], lhsT=yp0[:, r, :],
                                 rhs=at2[:, r, lo * P:hi * P],
                                 start=False, stop=(r == NT - 1),
                                 skip_group_check=True)
            # (b) strip r, weight blocks c <= r
            for c in range(r + 1):
                nc.tensor.matmul(z1[:, r * P:(r + 1) * P], lhsT=yp0[:, c, :],
                                 rhs=at2[:, c, r * P:(r + 1) * P],
                                 start=(r % 4 == 0 and c == 0),
                                 stop=(c == NT - 1), skip_group_check=True)

        # ---- streaming phase -------------------------------------------
        for r in range(NT):
            a_f = afp.tile([P, N], f32, tag="af")
            nc.sync.dma_start(out=a_f[:, :H], in_=adj[r * P:(r + 1) * P, :H])
            nc.sync.dma_start(out=a_f[:, H:], in_=adj[r * P:(r + 1) * P, H:])

            # fused fp32->bf16 cast + row-sum halves (DVE low, ACT high)
            a_t = abp.tile([P, N], bf16, tag="a")
            d_lo = small.tile([P, 1], f32, tag="dlo")
            d_hi = small.tile([P, 1], f32, tag="dhi")
            nc.vector.tensor_scalar(
                out=a_t[:, :H], in0=a_f[:, :H], scalar1=1.0, scalar2=0.0,
                op0=OP.mult, op1=OP.add, accum_out=d_lo)
            nc.scalar.activation(out=a_t[:, H:], in_=a_f[:, H:],
                                 func=AF.Identity, accum_out=d_hi)

            # transpose A strip r (no scaling -> only gated on the cast)
            for g in range(4):
                pt = ps_t.tile([P, 4, P], f32, tag="t")
                for q in range(4):
                    c = 4 * g + q
                    nc.tensor.matmul(pt[:, q, :], lhsT=a_t[:, c * P:(c + 1) * P],
                                     rhs=ident_bf, start=True, stop=True)
                if g % 2 == 0:
                    nc.vector.tensor_copy(
                        out=at2[:, 4 * g:4 * g + 4, r * P:(r + 1) * P], in_=pt)
                else:
                    nc.scalar.copy(
                        out=at2[:, 4 * g:4 * g + 4, r * P:(r + 1) * P], in_=pt)

            # scalar chain for strip r
            d_r = small.tile([P, 1], f32, tag="d")
            nc.vector.tensor_scalar(out=d_r, in0=d_lo, scalar1=d_hi,
                                    scalar2=None, op0=OP.add)
            dsq_r = small.tile([P, 1], f32, tag="dsq")
            nc.scalar.activation(out=dsq_r, in_=d_r, func=AF.Sqrt, bias=eps_sb)
            nc.vector.reciprocal(out=dinv[:, r:r + 1], in_=dsq_r)
            nc.vector.tensor_scalar(out=dinv2[:, r:r + 1], in0=dinv[:, r:r + 1],
                                    scalar1=dinv[:, r:r + 1], scalar2=None,
                                    op0=OP.mult)
            nc.vector.tensor_scalar(out=u0[:, r, :], in0=x_sb[:, r, :],
                                    scalar1=dsq_r, scalar2=None, op0=OP.mult)
            nc.vector.tensor_scalar(out=yp0[:, r, :], in0=x_sb[:, r, :],
                                    scalar1=dinv[:, r:r + 1], scalar2=None,
                                    op0=OP.mult)

            # PE work that depends on strip r-1's scalar chain, emitted here
            # so that strip r's transposes (ready as soon as the cast lands)
            # sit AHEAD of it in the PE queue -- avoids head-of-line stalls.
            if r >= 1:
                emit_u0T_and_pass1(r - 1)
        emit_u0T_and_pass1(NT - 1)

        # ---- recurrence: u1T = u0T - z1T (pure DVE, no scaling) --------
        for ch in range(4):
            s = slice(ch * 512, (ch + 1) * 512)
            nc.vector.scalar_tensor_tensor(
                out=uT1[:, s], in0=z1[:, s], scalar=-1.0, in1=uT0[:, s],
                op0=OP.mult, op1=OP.add)

        # ttT = 2*u1T - u0T (overlaps pass 2)
        for ch in range(4):
            s = slice(ch * 512, (ch + 1) * 512)
            nc.vector.scalar_tensor_tensor(
                out=ttT[:, s], in0=uT1[:, s], scalar=2.0, in1=uT0[:, s],
                op0=OP.mult, op1=OP.subtract)

        # ---- pass 2: y1s[c] = dinv2[c]*u1n[c] (scale fused in ACT copy),
        #      then wide MMs against at2 ---------------------------------
        z2 = ps_acc.tile([P, N], f32, tag="acc")
        for c in range(NT):
            ptc = ps_t.tile([P, 4, P], f32, tag="t")
            nc.tensor.matmul(ptc[:, 0, :], lhsT=uT1[:, c * P:(c + 1) * P],
                             rhs=ident_bf, start=True, stop=True)
            nc.scalar.mul(out=y1s[:, c, :], in_=ptc[:, 0, :],
                          mul=dinv2[:, c:c + 1])
            for nch in range(4):
                nc.tensor.matmul(z2[:, nch * 512:(nch + 1) * 512],
                                 lhsT=y1s[:, c, :],
                                 rhs=at2[:, c, nch * 512:(nch + 1) * 512],
                                 start=(c == 0), stop=(c == NT - 1))

        # u2T = ttT - 2*z2T (DVE)
        for ch in range(4):
            s = slice(ch * 512, (ch + 1) * 512)
            nc.vector.scalar_tensor_tensor(
                out=uT2[:, s], in0=z2[:, s], scalar=-2.0, in1=ttT[:, s],
                op0=OP.mult, op1=OP.add)

        # ---- output layer: out[i,:] = relu(dinv[i]*(sum_k uTk[:,i]^T Wk)+bsum)
        uTs = (uT0, uT1, uT2)
        for g in range(4):
            og = small.tile([P, 4, F], f32, tag="og")
            for q in range(4):
                rr = 4 * g + q
                oc = ps_t.tile([P, 4, P], f32, tag="t")
                for k3 in range(K):
                    nc.tensor.matmul(oc[:, 0, :],
                                     lhsT=uTs[k3][:, rr * P:(rr + 1) * P],
                                     rhs=w_bf[:, k3, :],
                                     start=(k3 == 0), stop=(k3 == K - 1))
                tmp = small.tile([P, F], f32, tag="tmp")
                nc.vector.scalar_tensor_tensor(
                    out=tmp, in0=oc[:, 0, :], scalar=dinv[:, rr:rr + 1],
                    in1=bsum, op0=OP.mult, op1=OP.add)
                nc.scalar.activation(out=og[:, q, :], in_=tmp, func=AF.Relu)
            nc.sync.dma_start(out=out_t[:, 4 * g:4 * g + 4, :], in_=og)

    nc.compile()
    return nc


def _get_nc():
    if "nc" not in _cache:
        _cache["nc"] = _build_nc()
    return _cache["nc"]


def make_in_maps(x, adj, W, b):
    import ml_dtypes

    bf16 = ml_dtypes.bfloat16
    identb = np.ascontiguousarray(np.eye(P, dtype=np.float32).astype(bf16))
    x = np.ascontiguousarray(np.asarray(x, dtype=np.float32))
    adj = np.ascontiguousarray(np.asarray(adj, dtype=np.float32))
    # W [K, in, out] -> [in, K, out] bf16 (host-side rearrange + cast)
    wb = np.ascontiguousarray(
        np.asarray(W, dtype=np.float32).transpose(1, 0, 2).astype(bf16))
    bf = np.asarray(b, dtype=np.float32)
    bsum = np.ascontiguousarray(
        np.broadcast_to(bf.sum(axis=0), (P, F)).astype(np.float32))
    return [
        {"adj": adj[c], "x": x[c], "wb": wb, "bsum": bsum, "identb": identb}
        for c in range(NCORES)
    ]


def run_raw(x, adj, W, b, **kwargs):
    from concourse import bass_utils

    nc = _get_nc()
    in_maps = make_in_maps(x, adj, W, b)
    res = bass_utils.run_bass_kernel_spmd(nc, in_maps,
                                          core_ids=list(range(NCORES)), **kwargs)
    out = np.stack([res.results[c]["out"] for c in range(NCORES)], axis=0)
    return out.astype(np.float32), res


def kernel(x, adj, W, b):
    out, _ = run_raw(x, adj, W, b)
    return out
